# revision 54
# baseline (speedup 1.0000x reference)
"""Trainium2 Bass kernel for ViTDet-style attention with decomposed relative
position bias (B=8, H=W=32, dim=768, 12 heads).

Strategy
--------
Data-parallel over the batch: each of the 8 NeuronCores processes one batch
element end-to-end (qkv projection, biased attention, output projection).

The decomposed rel-pos bias is folded into the QK^T matmul by augmenting the
per-head contraction dimension from 64 to exactly 128:
    K_aug = [ onehot_h (32) ; onehot_w (32) ; k^T (64) ]
    Q_aug = [ (q @ Rh)^T (32) ; (q @ Rw)^T (32) ; q^T (64) ]
so S^T = scale*(q.k) + rel_h + rel_w in ONE K=128 matmul per tile.

All matmul operands are bf16 (fast weight loads, half the DMA bytes); PSUM
accumulation stays fp32.  The logit scale folds 1/(8*16) into W_q so the
matmul produces S/16 directly:
 - the Scalar engine computes exp via ACTIVATE(Exp, scale=16)
 - the Vector engine computes exp via a custom 8-stage DVE op:
       exp16(z) = (1 + z*(c0 + z*c1))^16        (z = S/16, |z| <= ~0.19)
   a degree-2 seed + 4 squarings, max rel err ~0.5% over |S|<=3.
Splitting exp across both engines removes the Scalar-engine bottleneck that
limits an ACT-only softmax to ~128us.

Attention runs transposed (keys on partitions) so exp output feeds A@V with
no transposes.  V tiles carry 64 ones-columns next to the 64 value columns
(the matmul is stream-bound, so the extra stationary width is free): the A@V
accumulator then holds the softmax row-sums replicated across PSUM rows
0:64, and the reciprocal runs directly on those rows with a base-partition-0
DVE op -- no broadcast matmul, no 1-partition row copies.  (Custom DVE ops
at base partition 64 corrupt sporadically on hardware though they pass
CoreSim -- keep them at base 0.)  Scalar does the single cross-partition hop
(A@V PSUM rows 64:128 -> SBUF 0:64).

The rel_w bias matmul reads a w-major copy of q (built with strided Scalar
copies off the projection PSUM evacuation path) so its rhs streams
contiguously like rel_h; the strided-rhs version costs the PE ~4x per
instruction.

Bias handling (all exact):
 - k-bias adds a per-query constant to all key logits -> cancels in softmax.
 - v-bias and proj-bias contribute `qkv_b[v] @ proj_w + proj_b` to every
   output row (softmax rows sum to 1); added on the host after gather.
 - q-bias would need an extra device pass; inputs always have qkv_b == 0,
   but for full generality we fall back to an exact numpy path if nonzero.
"""

import functools
import os
import sys

import numpy as np

sys.path.insert(0, "/opt/trn_rl_repo")
os.environ.setdefault("MYCRO_LOCAL_CACHE", "1")

B, Hh, Ww, DIM = 8, 32, 32, 768
NH, HD = 12, 64
T = Hh * Ww  # 1024 tokens
KT = DIM // 128  # 6 contraction tiles
TT = T // 128  # 8 token tiles
N_CORES = 8

# exp16 seed coefficients (minimax for e^z on |z| <= 3/16, fp32-validated)
EXP16_C0 = 1.0042780
EXP16_C1 = 0.4998960
# every exp tile is split by columns across both engines: Scalar ACT takes
# cols 0:EXP_SPLIT, the Vector exp16 takes the rest.  This halves the exp
# LATENCY per tile (vs whole tiles alternating engines), which the 2-deep
# S-PSUM rotation needs: s(kt) WARs on exp(kt-2) being fully consumed.
EXP_SPLIT = 512

# module-level knobs (test.py pokes these)
TRACE = False
LAST = {}


@functools.lru_cache(maxsize=1)
def _exp16_op():
    """Register the custom DVE exp16 op via the documented extension point."""
    import concourse.dve_ops as dve_ops
    from concourse.dve_spec import C0, C1, One, Spec, Src0

    for op in dve_ops.OPS:
        if op.name == "EXP16_ANT":
            return op

    s = (Src0 * C1 + C0) * Src0 + One
    p = s * s
    p = p * p
    p = p * p
    body = p * p

    def ref(in0, in1, s0, s1, imm2):
        x = in0.astype(np.float32)
        t = ((x * np.float32(s1) + np.float32(s0)) * x + np.float32(1.0)).astype(
            np.float32
        )
        for _ in range(4):
            t = (t * t).astype(np.float32)
        return t

    op = dve_ops.DveOp(
        "EXP16_ANT",
        Spec(body=body, reference=ref),
        subdim=False,
        uops_sha={"v3": "3a278043e04e9b82", "v4": "aec3b4183f09a28e"},
    )
    dve_ops.OPS.append(op)
    dve_ops.CUSTOM_DVE_SPECS[op.name] = op.spec
    dve_ops._SUB_OPCODE_FOR_NAME[op.name] = (
        dve_ops._CUSTOM_DVE_ROW_BASE + len(dve_ops.OPS) - 1
    )
    assert dve_ops._SUB_OPCODE_FOR_NAME[op.name] < 0x20
    return op


@functools.lru_cache(maxsize=2)
def _build_program(dump: bool = False):
    """Emit the Bass/Tile program (identical on all 8 cores)."""
    from contextlib import ExitStack

    import concourse.bacc as bacc
    import concourse.tile as tile
    from concourse import mybir

    exp16 = _exp16_op()

    f32 = mybir.dt.float32
    bf16 = mybir.dt.bfloat16
    AF = mybir.ActivationFunctionType

    nc = bacc.Bacc("TRN2", target_bir_lowering=False, debug=False)

    xT = nc.dram_tensor("xT", [DIM, T], bf16, kind="ExternalInput").ap()
    wqk = nc.dram_tensor("wqk", [NH, 128, KT, 128], bf16, kind="ExternalInput").ap()
    wv = nc.dram_tensor("wv", [128, KT, DIM], bf16, kind="ExternalInput").ap()
    pw = nc.dram_tensor("pw", [128, KT, DIM], bf16, kind="ExternalInput").ap()
    onehot = nc.dram_tensor("onehot", [64, T], bf16, kind="ExternalInput").ap()
    relh = nc.dram_tensor("relh", [HD, Hh, Hh], bf16, kind="ExternalInput").ap()
    relw = nc.dram_tensor("relw", [HD, Ww, Ww], bf16, kind="ExternalInput").ap()
    y = nc.dram_tensor("y", [T, DIM], bf16, kind="ExternalOutput").ap()

    with tile.TileContext(nc) as tc, ExitStack() as ctx:
        persist = ctx.enter_context(tc.tile_pool(name="persist", bufs=1))
        xts = persist.tile([128, KT, T], bf16, tag="xts")
        # aug rows: 0:32 rel_h/onehot_h, 32:64 rel_w/onehot_w, 64:128 q|k
        qaug = persist.tile([128, NH, T], bf16, tag="qaug")
        kaug = persist.tile([128, NH, T], bf16, tag="kaug")
        # w-major copy of q (channels on partitions 0:64) for the rel_w matmul
        qwm = persist.tile([64, NH, Ww, Hh], bf16, tag="qwm")
        # v token-major; 64 ones-columns (cols 0:64) beside the 64 value
        # columns make the A@V accumulator carry softmax row-sums replicated
        # across psum partitions 0:64 with zero extra PE cost (stream-bound)
        vsb = persist.tile([128, TT, NH, 128], bf16, tag="vsb")
        # normalized per-head attention output, channel-major (proj lhsT)
        outT = persist.tile([128, KT, T], bf16, tag="outT")
        relhs = persist.tile([128, Hh, Hh], bf16, tag="relhs")
        relws = persist.tile([64, Ww, Ww], bf16, tag="relws")
        wqks = persist.tile([128, NH, KT, 128], bf16, tag="wqks")
        wvt = persist.tile([128, KT, DIM], bf16, tag="wvt")
        pwt = persist.tile([128, KT, DIM], bf16, tag="pwt")

        # ---------------- phase 0: input DMAs + memset --------------------
        # DMA order = critical path order: the first qk matmul needs
        # xts[kt=0] + wqks[h=0]; later kt tiles stream in while the PE works.
        # Each dma_start costs ~600-700ns of issuing-engine time, so the
        # first triggers on each engine gate everything behind them.
        # onehot/pw land during the PE-bound projection phase (DMA is idle
        # there), not in front of it.
        nc.sync.dma_start(out=xts[:, 0, 0:512], in_=xT[0:128, 0:512])
        nc.gpsimd.dma_start(out=wqks[:, 0], in_=wqk[0])
        nc.sync.dma_start(out=xts[:, 0, 512:1024], in_=xT[0:128, 512:1024])
        for kt in range(1, KT):
            eng = nc.sync if kt % 2 == 1 else nc.gpsimd
            eng.dma_start(out=xts[:, kt, :], in_=xT[kt * 128 : (kt + 1) * 128, :])
        nc.sync.dma_start(out=wvt, in_=wv)
        # last few wqks on the sync ring so the gpsimd ring isn't the only
        # one carrying 2.2MB of weights (wvt stays ahead of them: v_group
        # needs it ~16us in)
        for h in range(1, 9):
            nc.gpsimd.dma_start(out=wqks[:, h], in_=wqk[h])
        for h in range(9, NH):
            nc.sync.dma_start(out=wqks[:, h], in_=wqk[h])
        nc.sync.dma_start(out=relhs[64:128], in_=relh)
        nc.sync.dma_start(out=relws, in_=relw)
        for h in range(NH):
            nc.sync.dma_start(out=kaug[0:64, h, :], in_=onehot)
        nc.gpsimd.dma_start(out=pwt, in_=pw)
        # vsb ones columns 0:64 for every head.  Vector is idle until the
        # first kaug evac (~10us); on gpsimd this 5us memset would sit in
        # front of the wqks[0] trigger and stall the first matmul.
        nc.vector.memset(vsb[:, :, :, 0:64], 1.0)

        # ---------------- phase 1: q/k/v projection -----------------------
        with tc.tile_pool(name="ps_qk", bufs=2, space="PSUM") as ps_qk, \
             tc.tile_pool(name="ps_v", bufs=2, space="PSUM") as ps_v, \
             tc.tile_pool(name="ps_rel", bufs=2, space="PSUM") as ps_rel:

            def qk_group(h, n):
                ns = slice(n * 512, (n + 1) * 512)
                ps = ps_qk.tile([128, 512], f32, tag="qkps")
                for kt in range(KT):
                    nc.tensor.matmul(
                        ps,
                        lhsT=wqks[:, h, kt, :],
                        rhs=xts[:, kt, ns],
                        start=(kt == 0),
                        stop=(kt == KT - 1),
                    )
                # q rows cross partitions (Scalar), k rows aligned (Vector)
                nc.scalar.activation(qaug[64:128, h, ns], ps[0:64, :], AF.Identity)
                nc.vector.tensor_copy(kaug[64:128, h, ns], ps[64:128, :])

            def qwm_copy(h, n):
                # w-major strided copy of q for the rel_w rhs; Scalar crosses
                # partitions 64:128 -> 0:64, ~0.5us each, off critical path
                ns = slice(n * 512, (n + 1) * 512)
                src = qaug[64:128, h, ns].rearrange("p (hq w) -> p w hq", w=Ww)
                nc.scalar.activation(
                    qwm[:, h, :, 16 * n : 16 * (n + 1)], src, AF.Identity
                )

            def v_group(n, mt):
                ms = slice(mt * 128, (mt + 1) * 128)
                pv = ps_v.tile([128, 6, HD], f32, tag="vps")
                for kt in range(KT):
                    nc.tensor.matmul(
                        pv,
                        lhsT=xts[:, kt, ms],
                        rhs=wvt[:, kt, n * 384 : (n + 1) * 384],
                        start=(kt == 0),
                        stop=(kt == KT - 1),
                    )
                nc.vector.tensor_copy(vsb[:, mt, 6 * n : 6 * n + 6, 64:128], pv)

            # rel-pos bias rows: two blocks per bank-padded PSUM tile; both
            # rel_h and rel_w stream contiguous rhs now (qwm for rel_w)
            qw = qaug[32:64, :, :].rearrange("p h (q w) -> p h q w", w=Ww)

            # rel evac engine alternates (the ~0.9us strided evacs are what
            # bound the pure-rel tail; splitting them over both engines
            # doubles the drain rate of the 2-buffer ps_rel rotation)
            rel_evac = [0]

            def relh_pair(hh, eng=None):
                rp = ps_rel.tile([128, 2, 512], f32, tag="relps", name="rph")
                for j in range(2):
                    b = hh + j
                    nc.tensor.matmul(
                        rp[0:32, j, 0:384],
                        lhsT=relhs[64:128, b, :],
                        rhs=qaug[64:128, :, b * 32 : (b + 1) * 32],
                        start=True,
                        stop=True,
                    )
                rh_src = rp[0:32, :, 0:384].rearrange("p j (h q) -> p h j q", q=32)
                dst = qaug[0:32, :, hh * 32 : (hh + 2) * 32]
                if eng is None:
                    eng = "sv"[rel_evac[0] % 2]
                    rel_evac[0] += 1
                if eng == "s":
                    nc.scalar.activation(dst, rh_src, AF.Identity)
                else:
                    nc.vector.tensor_copy(dst, rh_src)

            def relw_pair(hh):
                rp = ps_rel.tile([128, 2, 512], f32, tag="relps", name="rpw")
                for j in range(2):
                    b = hh + j
                    nc.tensor.matmul(
                        rp[32:64, j, 0:384],
                        lhsT=relws[:, b, :],
                        rhs=qwm[:, :, b, :],
                        start=True,
                        stop=True,
                    )
                # evac split across BOTH engines (one block each, in
                # parallel): the ~0.9us whole-pair evac was what the 2-deep
                # ps_rel rotation waited on, stalling the PE in the rel tail
                for j in range(2):
                    src = rp[32:64, j : j + 1, 0:384].rearrange(
                        "p j (h q) -> p h q j", q=32
                    )
                    dst = qw[:, :, :, hh + j : hh + j + 1]
                    if (rel_evac[0] + j) % 2 == 0:
                        nc.scalar.activation(dst, src, AF.Identity)
                    else:
                        nc.vector.tensor_copy(dst, src)
                rel_evac[0] += 1

            vg = [(n, mt) for n in range(2) for mt in range(TT)]
            # first 4 heads do both column halves back-to-back: each early
            # head then needs one wqks arrival per 2.56us instead of 1.28us,
            # which the DMA rings can actually sustain at kernel start
            for h in range(4):
                qk_group(h, 0)
                qwm_copy(h, 0)
                qk_group(h, 1)
                qwm_copy(h, 1)
            # rest of pass n=0, v projection starting once wvt landed
            for h in range(4, NH):
                qk_group(h, 0)
                qwm_copy(h, 0)
                if h >= 8 and vg:
                    v_group(*vg.pop(0))
            # pass n=1 with rel-h pairs for query blocks 0..15 (all in n=0)
            # and more v projection interleaved
            relh_a = list(range(0, 16, 2))
            for h in range(4, NH):
                qk_group(h, 1)
                qwm_copy(h, 1)
                if relh_a:
                    # vector evac: scalar runs ~97% during the qk passes
                    # (q + qwm copies) while vector has slack
                    relh_pair(relh_a.pop(0), eng="v")
                if vg:
                    v_group(*vg.pop(0))

            # tail: all rel-w pairs + rest of v.  (rel_w block b covers
            # queries with w-coord b, which scatter over BOTH column halves,
            # so every relw block gates attention pass 0.  The rel-h blocks
            # 16..31 gate only pass 1 and are deferred into pass 0's stream.)
            for i in range(16):
                relw_pair(2 * i)
                if vg and i % 2 == 0:
                    v_group(*vg.pop(0))
            while vg:
                v_group(*vg.pop(0))

        # ------------- phase 2+3: attention + overlapped projection -------
        # Attention runs as two passes over query column halves (n=0 then
        # n=1).  After pass 0, outT[:, :, 0:512] is complete, so the output
        # projection for token blocks 0..3 interleaves into pass 1: its PE
        # matmuls act as gap filler and -- more importantly -- its PSUM
        # evacuations and the HBM output drain (~12us exposed otherwise; the
        # drain runs at only ~130 GB/s with 8 cores writing at once) overlap
        # attention compute.  Only token blocks 4..7 drain in the tail.
        from concourse.dve_ops import (
            RECIP_APPROX_FAST_CONSTS as _RC,
            RECIPROCAL_APPROX_FAST as _RF,
        )

        # whole-head normalization is deferred into the NEXT (h, n)
        # iteration's streams (safe: ps_av double-buffered).  A long Vector
        # op emitted at a head boundary stalls the next head's s-tiles
        # ~1.2us on hardware (coarse cross-engine semaphore thresholds), so
        # nothing norm-related may sit between the last A@V of one head and
        # the first exps of the next.
        deferred = [None]

        def emit_norm(norm, stage):
            h, ns, avps, uav, rb = norm
            rows = slice(0, 64) if h % 2 == 0 else slice(64, 128)
            if stage == 0:
                nc.scalar.activation(uav, avps[64:128], AF.Identity)
            elif stage == 1:
                nc.vector._custom_dve(
                    _RF, out=rb, in0=avps[0:64],
                    s0=_RC["s0"], s1=_RC["s1"], imm2=_RC["imm2"],
                )
            else:
                nc.vector.tensor_mul(outT[rows, h // 2, ns], uav, rb)

        with tc.tile_pool(name="pt", bufs=6) as ppt, \
             tc.tile_pool(name="rb", bufs=2) as prb, \
             tc.tile_pool(name="uavp", bufs=2) as puav, \
             tc.tile_pool(name="py", bufs=3) as py, \
             tc.tile_pool(name="ps_s", bufs=4, space="PSUM") as ps_s, \
             tc.tile_pool(name="ps_y", bufs=2, space="PSUM") as ps_y, \
             tc.tile_pool(name="ps_av", bufs=2, space="PSUM") as ps_av:

            # rel-h bias for query blocks 16..31 (needed only by pass 1),
            # interleaved into pass 0 as PE filler; PSUM comes from the
            # proj pool whose banks are idle until pass 1
            rel1_i = [0]

            def relh_single(b):
                # same tag+shape as the proj tiles: pool buffers are per-tag
                rp = ps_y.tile([128, 384], f32, tag="yps", name="rel1")
                nc.tensor.matmul(
                    rp[0:32, 0:384],
                    lhsT=relhs[64:128, b, :],
                    rhs=qaug[64:128, :, b * 32 : (b + 1) * 32],
                    start=True,
                    stop=True,
                )
                src = rp[0:32, 0:384].rearrange("p (h q) -> p h q", q=32)
                dst = qaug[0:32, :, b * 32 : (b + 1) * 32]
                if rel1_i[0] % 2 == 0:
                    nc.scalar.activation(dst, src, AF.Identity)
                else:
                    nc.vector.tensor_copy(dst, src)
                rel1_i[0] += 1

            def proj_mm(tiles, mt, kts):
                for kt in kts:
                    for j in range(2):
                        nc.tensor.matmul(
                            tiles[j],
                            lhsT=outT[:, kt, mt * 128 : (mt + 1) * 128],
                            rhs=pwt[:, kt, j * 384 : (j + 1) * 384],
                            start=(kt == 0),
                            stop=(kt == KT - 1),
                        )

            def proj_finish(tiles, mt, g):
                yt = py.tile([128, DIM], bf16, tag="yt")
                nc.scalar.activation(yt[:, 0:384], tiles[0], AF.Identity)
                nc.vector.tensor_copy(yt[:, 384:768], tiles[1])
                # ONE full-width contiguous DMA per tile: a single
                # InstDMACopy is split across all 16 SDMA slots of its ring,
                # while partition-chunked transfers SERIALIZE on the same
                # rings and each pays the long HBM-write completion latency.
                # Rotate the three DGE rings (SP / Act / SWDGE) so
                # consecutive tiles' completions overlap.
                eng = (nc.sync, nc.scalar, nc.gpsimd)[g % 3]
                eng.dma_start(out=y[mt * 128 : (mt + 1) * 128, :], in_=yt)

            def proj_group(mt, g):
                tiles = {
                    j: ps_y.tile([128, 384], f32, tag="yps", name=f"yps{g % 2}")
                    for j in range(2)
                }
                proj_mm(tiles, mt, range(KT))
                proj_finish(tiles, mt, g)

            for n in range(2):
                ns = slice(n * 512, (n + 1) * 512)
                # exp engine split per half: 4.5 Scalar / 3.5 Vector average
                dve_kt = (2, 5, 7) if n == 0 else (2, 3, 5, 7)
                for h in range(NH):
                    # allocated lazily at first use: allocating earlier
                    # attaches the buffer-reuse WAR to the next PE
                    # instruction emitted and stalls the s-tiles
                    avps = [None]

                    def s_tile(kt):
                        sp = ps_s.tile([128, 512], f32, tag="sps", name="sp")
                        nc.tensor.matmul(
                            sp,
                            lhsT=kaug[:, h, kt * 128 : (kt + 1) * 128],
                            rhs=qaug[:, h, ns],
                            start=True,
                            stop=True,
                        )
                        return sp

                    def exp_tile(kt, sp):
                        pt = ppt.tile([128, 512], bf16, tag="pt")
                        if kt in dve_kt:
                            nc.vector._custom_dve(
                                exp16, out=pt, in0=sp, s0=EXP16_C0, s1=EXP16_C1
                            )
                        else:
                            nc.scalar.activation(pt, sp, AF.Exp, scale=16.0)
                        return pt

                    def av_tile(kt, pt):
                        if avps[0] is None:
                            avps[0] = ps_av.tile(
                                [128, 512], f32, tag="avps", name="avps"
                            )
                        nc.tensor.matmul(
                            avps[0],
                            lhsT=vsb[:, kt, h, :],
                            rhs=pt,
                            start=(kt == 0),
                            stop=(kt == TT - 1),
                        )

                    pts = [
                        exp_tile(0, s_tile(0)),
                        exp_tile(1, s_tile(1)),
                        exp_tile(2, s_tile(2)),
                    ]
                    for kt in range(3, TT):
                        sp2 = s_tile(kt)
                        av_tile(kt - 3, pts.pop(0))
                        if deferred[0] is not None and 3 <= kt <= 5:
                            emit_norm(deferred[0], kt - 3)
                            if kt == 5:
                                deferred[0] = None
                        pts.append(exp_tile(kt, sp2))
                    for j in range(3):
                        av_tile(TT - 3 + j, pts.pop(0))

                    # avps rows 0:64 hold the softmax row-sums replicated
                    # x64 (ones columns of vsb), rows 64:128 hold A@V.
                    # Scalar does the one cross-partition hop (PSUM 64:128
                    # -> SBUF 0:64); the reciprocal runs at base partition 0
                    # (custom DVE at base 64 corrupts on hardware).
                    uav = puav.tile([64, 512], bf16, tag="uav")
                    rb = prb.tile([64, 512], bf16, tag="rb")
                    deferred[0] = (h, ns, avps[0], uav, rb)

                    # pass 0: deferred rel-h singles as filler (2 per head)
                    if n == 0 and 1 <= h <= 8:
                        relh_single(14 + 2 * h)
                        relh_single(15 + 2 * h)
                    # overlapped projection for pass-0 token blocks, spread
                    # through pass 1 (outT cols 0:512 complete once the
                    # deferred norm of (h=11, n=0) lands during h=0 here);
                    # mt4 pre-accumulates its first 5 head pairs at h=10 so
                    # only its kt=5 matmul waits on the final norm
                    if n == 1 and h in (1, 4, 6, 8):
                        gi = {1: 0, 4: 1, 6: 2, 8: 3}[h]
                        proj_group(mt=gi, g=gi)
                    if n == 1 and h == 10:
                        mt4_tiles = {
                            j: ps_y.tile([128, 384], f32, tag="yps", name="yps4")
                            for j in range(2)
                        }
                        proj_mm(mt4_tiles, 4, range(5))

            # final head's norm: pipelined in column halves
            h, ns_l, avps_l, uav, rb = deferred[0]
            rows = slice(0, 64) if h % 2 == 0 else slice(64, 128)
            for cs, co in ((slice(0, 256), slice(512, 768)),
                           (slice(256, 512), slice(768, 1024))):
                nc.scalar.activation(uav[:, cs], avps_l[64:128, cs], AF.Identity)
                nc.vector._custom_dve(
                    _RF, out=rb[:, cs], in0=avps_l[0:64, cs],
                    s0=_RC["s0"], s1=_RC["s1"], imm2=_RC["imm2"],
                )
                nc.vector.tensor_mul(outT[rows, h // 2, co], uav[:, cs], rb[:, cs])

            # remaining projection: finish mt4, then the last three blocks
            proj_mm(mt4_tiles, 4, range(5, KT))
            proj_finish(mt4_tiles, 4, 4)
            for g, mt in enumerate(range(5, 8)):
                proj_group(mt, 5 + g)

        if dump:
            d_qaug = nc.dram_tensor("d_qaug", [128, NH, T], bf16, kind="ExternalOutput").ap()
            d_kaug = nc.dram_tensor("d_kaug", [128, NH, T], bf16, kind="ExternalOutput").ap()
            d_vsb = nc.dram_tensor("d_vsb", [128, TT, NH, 128], bf16, kind="ExternalOutput").ap()
            d_outT = nc.dram_tensor("d_outT", [128, KT, T], bf16, kind="ExternalOutput").ap()
            nc.sync.dma_start(out=d_qaug, in_=qaug)
            nc.sync.dma_start(out=d_kaug, in_=kaug)
            nc.sync.dma_start(out=d_vsb, in_=vsb)
            nc.sync.dma_start(out=d_outT, in_=outT)

    nc.compile()
    return nc


def _host_consts(qkv_w, proj_w, rel_pos_h, rel_pos_w):
    import ml_dtypes

    bf = ml_dtypes.bfloat16
    f = np.float32
    qs = f(0.125 / 16.0)  # logit scale folded so PSUM holds S/16

    Wq = qkv_w[:, 0:DIM] * qs
    Wk = qkv_w[:, DIM : 2 * DIM]
    wqk = np.empty((NH, 128, KT, 128), dtype=f)
    for h in range(NH):
        m = np.concatenate(
            [Wq[:, h * HD : (h + 1) * HD], Wk[:, h * HD : (h + 1) * HD]], axis=1
        )  # [768, 128]
        wqk[h] = m.reshape(KT, 128, 128).transpose(1, 0, 2)

    wv = np.ascontiguousarray(
        qkv_w[:, 2 * DIM : 3 * DIM].reshape(KT, 128, DIM).transpose(1, 0, 2), dtype=f
    )
    pwt = np.ascontiguousarray(
        proj_w.reshape(KT, 128, DIM).transpose(1, 0, 2), dtype=f
    )

    k_idx = np.arange(T)
    onehot = np.zeros((64, T), dtype=f)
    onehot[k_idx // Ww, k_idx] = 1.0  # rows 0:32  -> h one-hot
    onehot[32 + (k_idx % Ww), k_idx] = 1.0  # rows 32:64 -> w one-hot

    # relh[c, hq, i] = 8 * rel_pos_h[hq - i + (Hh-1), c]; with q scaled by
    # 0.125/16 the matmul yields rel_h/16 exactly like the qk part.
    hq = np.arange(Hh)[:, None]
    ii = np.arange(Hh)[None, :]
    relh = (8.0 * rel_pos_h[(hq - ii + Hh - 1)]).transpose(2, 0, 1)
    relw = (8.0 * rel_pos_w[(hq - ii + Ww - 1)]).transpose(2, 0, 1)
    return {
        "wqk": wqk.astype(bf),
        "wv": wv.astype(bf),
        "pw": pwt.astype(bf),
        "onehot": onehot.astype(bf),
        "relh": np.ascontiguousarray(relh, dtype=f).astype(bf),
        "relw": np.ascontiguousarray(relw, dtype=f).astype(bf),
    }


def _numpy_reference(x, qkv_w, qkv_b, proj_w, proj_b, rel_pos_h, rel_pos_w):
    """Exact fallback (only used if qkv_b's q-part is nonzero)."""
    b, h, w, dim = x.shape
    hw = h * w
    scale = HD ** -0.5
    qkv = x.reshape(b, hw, dim) @ qkv_w + qkv_b
    qkv = qkv.reshape(b, hw, 3, NH, HD).transpose(2, 0, 3, 1, 4)
    qkv = qkv.reshape(3, b * NH, hw, HD)
    q, k, v = qkv[0], qkv[1], qkv[2]
    idx_h = np.arange(h)[:, None] - np.arange(h)[None, :] + (h - 1)
    idx_w = np.arange(w)[:, None] - np.arange(w)[None, :] + (w - 1)
    Rh = rel_pos_h[idx_h]
    Rw = rel_pos_w[idx_w]
    r_q = q.reshape(b * NH, h, w, HD)
    rel_h = np.einsum("bhwc,hkc->bhwk", r_q, Rh)
    rel_w = np.einsum("bhwc,wkc->bhwk", r_q, Rw)
    bias = (rel_h[:, :, :, :, None] + rel_w[:, :, :, None, :]).reshape(
        b * NH, hw, hw
    )
    attn = np.einsum("bqd,bkd->bqk", q, k) * scale + bias
    attn = attn - attn.max(axis=-1, keepdims=True)
    attn = np.exp(attn)
    attn /= attn.sum(axis=-1, keepdims=True)
    out = np.einsum("bqk,bkd->bqd", attn, v)
    out = out.reshape(b, NH, h, w, HD).transpose(0, 2, 3, 1, 4).reshape(b, h, w, dim)
    return (out @ proj_w + proj_b).astype(np.float32)


def kernel(x, qkv_w, qkv_b, proj_w, proj_b, rel_pos_h, rel_pos_w):
    import ml_dtypes

    bf = ml_dtypes.bfloat16
    x = np.asarray(x, dtype=np.float32)
    qkv_w = np.asarray(qkv_w, dtype=np.float32)
    qkv_b = np.asarray(qkv_b, dtype=np.float32)
    proj_w = np.asarray(proj_w, dtype=np.float32)
    proj_b = np.asarray(proj_b, dtype=np.float32)
    rel_pos_h = np.asarray(rel_pos_h, dtype=np.float32)
    rel_pos_w = np.asarray(rel_pos_w, dtype=np.float32)

    if np.any(qkv_b[0:DIM] != 0.0):
        # exact general fallback; never hit for this problem's inputs
        return _numpy_reference(
            x, qkv_w, qkv_b, proj_w, proj_b, rel_pos_h, rel_pos_w
        )

    from concourse.bass_utils import run_bass_kernel_spmd

    nc = _build_program()
    consts = _host_consts(qkv_w, proj_w, rel_pos_h, rel_pos_w)
    in_maps = []
    for b in range(B):
        m = dict(consts)
        m["xT"] = np.ascontiguousarray(x[b].reshape(T, DIM).T).astype(bf)
        in_maps.append(m)

    res = run_bass_kernel_spmd(nc, in_maps, list(range(N_CORES)), trace=TRACE)
    LAST["exec_time_ns"] = res.exec_time_ns
    LAST["results"] = res
    out = np.stack(
        [res.results[b]["y"].astype(np.float32).reshape(Hh, Ww, DIM) for b in range(B)]
    )

    # v-bias + proj-bias contribution (exact; softmax rows sum to 1)
    host_bias = qkv_b[2 * DIM : 3 * DIM] @ proj_w + proj_b
    if np.any(host_bias != 0.0):
        out = out + host_bias.astype(np.float32)
    return out.astype(np.float32, copy=False)


# revision 56
# speedup vs baseline: 1.3496x; 1.3496x over previous
"""Trainium2 Bass kernel for ViTDet-style attention with decomposed relative
position bias (B=8, H=W=32, dim=768, 12 heads).

Strategy
--------
Data-parallel over the batch: each of the 8 NeuronCores processes one batch
element end-to-end (qkv projection, biased attention, output projection).

The decomposed rel-pos bias is folded into the QK^T matmul by augmenting the
per-head contraction dimension from 64 to exactly 128:
    K_aug = [ onehot_h (32) ; onehot_w (32) ; k^T (64) ]
    Q_aug = [ (q @ Rh)^T (32) ; (q @ Rw)^T (32) ; q^T (64) ]
so S^T = scale*(q.k) + rel_h + rel_w in ONE K=128 matmul per tile.

All matmul operands are bf16 (fast weight loads, half the DMA bytes); PSUM
accumulation stays fp32.  The logit scale folds 1/(8*16) into W_q so the
matmul produces S/16 directly:
 - the Scalar engine computes exp via ACTIVATE(Exp, scale=16)
 - the Vector engine computes exp via a custom 8-stage DVE op:
       exp16(z) = (1 + z*(c0 + z*c1))^16        (z = S/16, |z| <= ~0.19)
   a degree-2 seed + 4 squarings, max rel err ~0.5% over |S|<=3.
Splitting exp across both engines removes the Scalar-engine bottleneck that
limits an ACT-only softmax to ~128us.

Attention runs transposed (keys on partitions) so exp output feeds A@V with
no transposes.  V tiles carry 64 ones-columns next to the 64 value columns
(the matmul is stream-bound, so the extra stationary width is free): the A@V
accumulator then holds the softmax row-sums replicated across PSUM rows
0:64, and the reciprocal runs directly on those rows with a base-partition-0
DVE op -- no broadcast matmul, no 1-partition row copies.  (Custom DVE ops
at base partition 64 corrupt sporadically on hardware though they pass
CoreSim -- keep them at base 0.)  Scalar does the single cross-partition hop
(A@V PSUM rows 64:128 -> SBUF 0:64).

The rel_w bias matmul reads a w-major copy of q (built with strided Scalar
copies off the projection PSUM evacuation path) so its rhs streams
contiguously like rel_h; the strided-rhs version costs the PE ~4x per
instruction.

Bias handling (all exact):
 - k-bias adds a per-query constant to all key logits -> cancels in softmax.
 - v-bias and proj-bias contribute `qkv_b[v] @ proj_w + proj_b` to every
   output row (softmax rows sum to 1); added on the host after gather.
 - q-bias would need an extra device pass; inputs always have qkv_b == 0,
   but for full generality we fall back to an exact numpy path if nonzero.
"""

import functools
import os
import sys

import numpy as np

sys.path.insert(0, "/opt/trn_rl_repo")
os.environ.setdefault("MYCRO_LOCAL_CACHE", "1")

B, Hh, Ww, DIM = 8, 32, 32, 768
NH, HD = 12, 64
T = Hh * Ww  # 1024 tokens
KT = DIM // 128  # 6 contraction tiles
TT = T // 128  # 8 token tiles
N_CORES = 8

# exp16 seed coefficients (minimax for e^z on |z| <= 3/16, fp32-validated)
EXP16_C0 = 1.0042780
EXP16_C1 = 0.4998960
# every exp tile is split by columns across both engines: Scalar ACT takes
# cols 0:EXP_SPLIT, the Vector exp16 takes the rest.  This halves the exp
# LATENCY per tile (vs whole tiles alternating engines), which the 2-deep
# S-PSUM rotation needs: s(kt) WARs on exp(kt-2) being fully consumed.
EXP_SPLIT = 512

# module-level knobs (test.py pokes these)
TRACE = False
LAST = {}


@functools.lru_cache(maxsize=1)
def _exp16_op():
    """Register the custom DVE exp16 op via the documented extension point."""
    import concourse.dve_ops as dve_ops
    from concourse.dve_spec import C0, C1, One, Spec, Src0

    for op in dve_ops.OPS:
        if op.name == "EXP16_ANT":
            return op

    s = (Src0 * C1 + C0) * Src0 + One
    p = s * s
    p = p * p
    p = p * p
    body = p * p

    def ref(in0, in1, s0, s1, imm2):
        x = in0.astype(np.float32)
        t = ((x * np.float32(s1) + np.float32(s0)) * x + np.float32(1.0)).astype(
            np.float32
        )
        for _ in range(4):
            t = (t * t).astype(np.float32)
        return t

    op = dve_ops.DveOp(
        "EXP16_ANT",
        Spec(body=body, reference=ref),
        subdim=False,
        uops_sha={"v3": "3a278043e04e9b82", "v4": "aec3b4183f09a28e"},
    )
    dve_ops.OPS.append(op)
    dve_ops.CUSTOM_DVE_SPECS[op.name] = op.spec
    dve_ops._SUB_OPCODE_FOR_NAME[op.name] = (
        dve_ops._CUSTOM_DVE_ROW_BASE + len(dve_ops.OPS) - 1
    )
    assert dve_ops._SUB_OPCODE_FOR_NAME[op.name] < 0x20
    return op


@functools.lru_cache(maxsize=2)
def _build_program(dump: bool = False):
    """Emit the Bass/Tile program (identical on all 8 cores)."""
    from contextlib import ExitStack

    import concourse.bacc as bacc
    import concourse.tile as tile
    from concourse import mybir

    exp16 = _exp16_op()

    f32 = mybir.dt.float32
    bf16 = mybir.dt.bfloat16
    AF = mybir.ActivationFunctionType

    nc = bacc.Bacc("TRN2", target_bir_lowering=False, debug=False)

    xT = nc.dram_tensor("xT", [DIM, T], bf16, kind="ExternalInput").ap()
    wqk = nc.dram_tensor("wqk", [NH, 128, KT, 128], bf16, kind="ExternalInput").ap()
    wv = nc.dram_tensor("wv", [128, KT, DIM], bf16, kind="ExternalInput").ap()
    pw = nc.dram_tensor("pw", [128, KT, DIM], bf16, kind="ExternalInput").ap()
    onehot = nc.dram_tensor("onehot", [64, T], bf16, kind="ExternalInput").ap()
    relh = nc.dram_tensor("relh", [HD, Hh, Hh], bf16, kind="ExternalInput").ap()
    relw = nc.dram_tensor("relw", [HD, Ww, Ww], bf16, kind="ExternalInput").ap()
    y = nc.dram_tensor("y", [T, DIM], bf16, kind="ExternalOutput").ap()

    with tile.TileContext(nc) as tc, ExitStack() as ctx:
        persist = ctx.enter_context(tc.tile_pool(name="persist", bufs=1))
        xts = persist.tile([128, KT, T], bf16, tag="xts")
        # aug rows: 0:32 rel_h/onehot_h, 32:64 rel_w/onehot_w, 64:128 q|k
        qaug = persist.tile([128, NH, T], bf16, tag="qaug")
        kaug = persist.tile([128, NH, T], bf16, tag="kaug")
        # w-major copy of q (channels on partitions 0:64) for the rel_w matmul
        qwm = persist.tile([64, NH, Ww, Hh], bf16, tag="qwm")
        # v token-major; 64 ones-columns (cols 0:64) beside the 64 value
        # columns make the A@V accumulator carry softmax row-sums replicated
        # across psum partitions 0:64 with zero extra PE cost (stream-bound)
        vsb = persist.tile([128, TT, NH, 128], bf16, tag="vsb")
        # normalized per-head attention output, channel-major (proj lhsT)
        outT = persist.tile([128, KT, T], bf16, tag="outT")
        relhs = persist.tile([128, Hh, Hh], bf16, tag="relhs")
        relws = persist.tile([64, Ww, Ww], bf16, tag="relws")
        wqks = persist.tile([128, NH, KT, 128], bf16, tag="wqks")
        wvt = persist.tile([128, KT, DIM], bf16, tag="wvt")
        pwt = persist.tile([128, KT, DIM], bf16, tag="pwt")

        # ---------------- phase 0: input DMAs + memset --------------------
        # DMA order = critical path order: the first qk matmul needs
        # xts[kt=0] + wqks[h=0]; later kt tiles stream in while the PE works.
        # Each dma_start costs ~600-700ns of issuing-engine time, so the
        # first triggers on each engine gate everything behind them.
        # onehot/pw land during the PE-bound projection phase (DMA is idle
        # there), not in front of it.
        nc.sync.dma_start(out=xts[:, 0, 0:512], in_=xT[0:128, 0:512])
        nc.gpsimd.dma_start(out=wqks[:, 0], in_=wqk[0])
        nc.sync.dma_start(out=xts[:, 0, 512:1024], in_=xT[0:128, 512:1024])
        # half-tile granularity alternating rings: the early qk groups wait
        # on bursty whole-kt arrivals otherwise (~1.2-1.5us gaps at t=12-20us)
        for kt in range(1, KT):
            for c in range(2):
                eng = nc.sync if (2 * kt + c) % 2 == 1 else nc.gpsimd
                cs = slice(c * 512, (c + 1) * 512)
                eng.dma_start(
                    out=xts[:, kt, cs], in_=xT[kt * 128 : (kt + 1) * 128, cs]
                )
        nc.sync.dma_start(out=wvt, in_=wv)
        # last few wqks on the sync ring so the gpsimd ring isn't the only
        # one carrying 2.2MB of weights (wvt stays ahead of them: v_group
        # needs it ~16us in)
        for h in range(1, 9):
            nc.gpsimd.dma_start(out=wqks[:, h], in_=wqk[h])
        for h in range(9, NH):
            nc.sync.dma_start(out=wqks[:, h], in_=wqk[h])
        nc.sync.dma_start(out=relhs[64:128], in_=relh)
        nc.sync.dma_start(out=relws, in_=relw)
        for h in range(NH):
            nc.sync.dma_start(out=kaug[0:64, h, :], in_=onehot)
        nc.gpsimd.dma_start(out=pwt, in_=pw)
        # vsb ones columns 0:64 for every head.  Vector is idle until the
        # first kaug evac (~10us); on gpsimd this 5us memset would sit in
        # front of the wqks[0] trigger and stall the first matmul.
        nc.vector.memset(vsb[:, :, :, 0:64], 1.0)

        # ---------------- phase 1: q/k/v projection -----------------------
        with tc.tile_pool(name="ps_qk", bufs=2, space="PSUM") as ps_qk, \
             tc.tile_pool(name="ps_v", bufs=2, space="PSUM") as ps_v, \
             tc.tile_pool(name="ps_rel", bufs=2, space="PSUM") as ps_rel:

            def qk_group(h, n):
                ns = slice(n * 512, (n + 1) * 512)
                ps = ps_qk.tile([128, 512], f32, tag="qkps")
                for kt in range(KT):
                    nc.tensor.matmul(
                        ps,
                        lhsT=wqks[:, h, kt, :],
                        rhs=xts[:, kt, ns],
                        start=(kt == 0),
                        stop=(kt == KT - 1),
                    )
                # q rows cross partitions (Scalar), k rows aligned (Vector)
                nc.scalar.activation(qaug[64:128, h, ns], ps[0:64, :], AF.Identity)
                nc.vector.tensor_copy(kaug[64:128, h, ns], ps[64:128, :])

            def qwm_copy(h, n):
                # w-major strided copy of q for the rel_w rhs; Scalar crosses
                # partitions 64:128 -> 0:64, ~0.5us each, off critical path
                ns = slice(n * 512, (n + 1) * 512)
                src = qaug[64:128, h, ns].rearrange("p (hq w) -> p w hq", w=Ww)
                nc.scalar.activation(
                    qwm[:, h, :, 16 * n : 16 * (n + 1)], src, AF.Identity
                )

            def v_group(n, mt):
                ms = slice(mt * 128, (mt + 1) * 128)
                pv = ps_v.tile([128, 6, HD], f32, tag="vps")
                for kt in range(KT):
                    nc.tensor.matmul(
                        pv,
                        lhsT=xts[:, kt, ms],
                        rhs=wvt[:, kt, n * 384 : (n + 1) * 384],
                        start=(kt == 0),
                        stop=(kt == KT - 1),
                    )
                nc.vector.tensor_copy(vsb[:, mt, 6 * n : 6 * n + 6, 64:128], pv)

            # rel-pos bias rows: two blocks per bank-padded PSUM tile; both
            # rel_h and rel_w stream contiguous rhs now (qwm for rel_w)
            qw = qaug[32:64, :, :].rearrange("p h (q w) -> p h q w", w=Ww)

            # rel evac engine alternates (the ~0.9us strided evacs are what
            # bound the pure-rel tail; splitting them over both engines
            # doubles the drain rate of the 2-buffer ps_rel rotation)
            rel_evac = [0]

            def relh_pair(hh, eng=None):
                rp = ps_rel.tile([128, 2, 512], f32, tag="relps", name="rph")
                for j in range(2):
                    b = hh + j
                    nc.tensor.matmul(
                        rp[0:32, j, 0:384],
                        lhsT=relhs[64:128, b, :],
                        rhs=qaug[64:128, :, b * 32 : (b + 1) * 32],
                        start=True,
                        stop=True,
                    )
                rh_src = rp[0:32, :, 0:384].rearrange("p j (h q) -> p h j q", q=32)
                dst = qaug[0:32, :, hh * 32 : (hh + 2) * 32]
                if eng is None:
                    eng = "sv"[rel_evac[0] % 2]
                    rel_evac[0] += 1
                if eng == "s":
                    nc.scalar.activation(dst, rh_src, AF.Identity)
                else:
                    nc.vector.tensor_copy(dst, rh_src)

            def relw_pair(hh):
                rp = ps_rel.tile([128, 2, 512], f32, tag="relps", name="rpw")
                for j in range(2):
                    b = hh + j
                    nc.tensor.matmul(
                        rp[32:64, j, 0:384],
                        lhsT=relws[:, b, :],
                        rhs=qwm[:, :, b, :],
                        start=True,
                        stop=True,
                    )
                rw_src = rp[32:64, :, 0:384].rearrange("p j (h q) -> p h q j", q=32)
                if "sv"[rel_evac[0] % 2] == "s":
                    nc.scalar.activation(
                        qw[:, :, :, hh : hh + 2], rw_src, AF.Identity
                    )
                else:
                    nc.vector.tensor_copy(qw[:, :, :, hh : hh + 2], rw_src)
                rel_evac[0] += 1

            vg = [(n, mt) for n in range(2) for mt in range(TT)]
            # first 4 heads do both column halves back-to-back: each early
            # head then needs one wqks arrival per 2.56us instead of 1.28us,
            # which the DMA rings can actually sustain at kernel start
            for h in range(4):
                qk_group(h, 0)
                qwm_copy(h, 0)
                qk_group(h, 1)
                qwm_copy(h, 1)
            # rest of pass n=0, v projection starting once wvt landed
            for h in range(4, NH):
                qk_group(h, 0)
                qwm_copy(h, 0)
                if h >= 8 and vg:
                    v_group(*vg.pop(0))
            # pass n=1 with rel-h pairs for query blocks 0..15 (all in n=0)
            # and more v projection interleaved
            relh_a = list(range(0, 16, 2))
            for h in range(4, NH):
                qk_group(h, 1)
                qwm_copy(h, 1)
                if relh_a:
                    # vector evac: scalar runs ~97% during the qk passes
                    # (q + qwm copies) while vector has slack
                    relh_pair(relh_a.pop(0), eng="v")
                if vg:
                    v_group(*vg.pop(0))

            # tail: all rel-w pairs + rest of v.  (rel_w block b covers
            # queries with w-coord b, which scatter over BOTH column halves,
            # so every relw block gates attention pass 0.  The rel-h blocks
            # 16..31 gate only pass 1 and are deferred into pass 0's stream.)
            for i in range(16):
                relw_pair(2 * i)
                if vg and i % 2 == 0:
                    v_group(*vg.pop(0))
            while vg:
                v_group(*vg.pop(0))

        # ------------- phase 2+3: attention + overlapped projection -------
        # Attention runs as two passes over query column halves (n=0 then
        # n=1).  After pass 0, outT[:, :, 0:512] is complete, so the output
        # projection for token blocks 0..3 interleaves into pass 1: its PE
        # matmuls act as gap filler and -- more importantly -- its PSUM
        # evacuations and the HBM output drain (~12us exposed otherwise; the
        # drain runs at only ~130 GB/s with 8 cores writing at once) overlap
        # attention compute.  Only token blocks 4..7 drain in the tail.
        from concourse.dve_ops import (
            RECIP_APPROX_FAST_CONSTS as _RC,
            RECIPROCAL_APPROX_FAST as _RF,
        )

        # whole-head normalization is deferred into the NEXT (h, n)
        # iteration's streams (safe: ps_av double-buffered).  A long Vector
        # op emitted at a head boundary stalls the next head's s-tiles
        # ~1.2us on hardware (coarse cross-engine semaphore thresholds), so
        # nothing norm-related may sit between the last A@V of one head and
        # the first exps of the next.
        deferred = [None]

        def emit_norm(norm, stage):
            h, ns, avps, uav, rb = norm
            rows = slice(0, 64) if h % 2 == 0 else slice(64, 128)
            if stage == 0:
                nc.scalar.activation(uav, avps[64:128], AF.Identity)
            elif stage == 1:
                nc.vector._custom_dve(
                    _RF, out=rb, in0=avps[0:64],
                    s0=_RC["s0"], s1=_RC["s1"], imm2=_RC["imm2"],
                )
            else:
                nc.vector.tensor_mul(outT[rows, h // 2, ns], uav, rb)

        with tc.tile_pool(name="pt", bufs=6) as ppt, \
             tc.tile_pool(name="rb", bufs=2) as prb, \
             tc.tile_pool(name="uavp", bufs=2) as puav, \
             tc.tile_pool(name="py", bufs=3) as py, \
             tc.tile_pool(name="ps_s", bufs=4, space="PSUM") as ps_s, \
             tc.tile_pool(name="ps_y", bufs=2, space="PSUM") as ps_y, \
             tc.tile_pool(name="ps_av", bufs=2, space="PSUM") as ps_av:

            # rel-h bias for query blocks 16..31 (needed only by pass 1),
            # interleaved into pass 0 as PE filler; PSUM comes from the
            # proj pool whose banks are idle until pass 1
            rel1_i = [0]

            def relh_single(b):
                # same tag+shape as the proj tiles: pool buffers are per-tag
                rp = ps_y.tile([128, 384], f32, tag="yps", name="rel1")
                nc.tensor.matmul(
                    rp[0:32, 0:384],
                    lhsT=relhs[64:128, b, :],
                    rhs=qaug[64:128, :, b * 32 : (b + 1) * 32],
                    start=True,
                    stop=True,
                )
                src = rp[0:32, 0:384].rearrange("p (h q) -> p h q", q=32)
                dst = qaug[0:32, :, b * 32 : (b + 1) * 32]
                if rel1_i[0] % 2 == 0:
                    nc.scalar.activation(dst, src, AF.Identity)
                else:
                    nc.vector.tensor_copy(dst, src)
                rel1_i[0] += 1

            def proj_mm(tiles, mt, kts):
                for kt in kts:
                    for j in range(2):
                        nc.tensor.matmul(
                            tiles[j],
                            lhsT=outT[:, kt, mt * 128 : (mt + 1) * 128],
                            rhs=pwt[:, kt, j * 384 : (j + 1) * 384],
                            start=(kt == 0),
                            stop=(kt == KT - 1),
                        )

            def proj_finish(tiles, mt, g):
                yt = py.tile([128, DIM], bf16, tag="yt")
                nc.scalar.activation(yt[:, 0:384], tiles[0], AF.Identity)
                nc.vector.tensor_copy(yt[:, 384:768], tiles[1])
                # ONE full-width contiguous DMA per tile: a single
                # InstDMACopy is split across all 16 SDMA slots of its ring,
                # while partition-chunked transfers SERIALIZE on the same
                # rings and each pays the long HBM-write completion latency.
                # Rotate the three DGE rings (SP / Act / SWDGE) so
                # consecutive tiles' completions overlap.
                eng = (nc.sync, nc.scalar, nc.gpsimd)[g % 3]
                eng.dma_start(out=y[mt * 128 : (mt + 1) * 128, :], in_=yt)

            def proj_group(mt, g):
                tiles = {
                    j: ps_y.tile([128, 384], f32, tag="yps", name=f"yps{g % 2}")
                    for j in range(2)
                }
                proj_mm(tiles, mt, range(KT))
                proj_finish(tiles, mt, g)

            for n in range(2):
                ns = slice(n * 512, (n + 1) * 512)
                # exp engine split per half: 4.5 Scalar / 3.5 Vector average
                dve_kt = (2, 5, 7) if n == 0 else (2, 3, 5, 7)
                for h in range(NH):
                    # allocated lazily at first use: allocating earlier
                    # attaches the buffer-reuse WAR to the next PE
                    # instruction emitted and stalls the s-tiles
                    avps = [None]

                    def s_tile(kt):
                        sp = ps_s.tile([128, 512], f32, tag="sps", name="sp")
                        nc.tensor.matmul(
                            sp,
                            lhsT=kaug[:, h, kt * 128 : (kt + 1) * 128],
                            rhs=qaug[:, h, ns],
                            start=True,
                            stop=True,
                        )
                        return sp

                    def exp_tile(kt, sp):
                        pt = ppt.tile([128, 512], bf16, tag="pt")
                        if kt in dve_kt:
                            nc.vector._custom_dve(
                                exp16, out=pt, in0=sp, s0=EXP16_C0, s1=EXP16_C1
                            )
                        else:
                            nc.scalar.activation(pt, sp, AF.Exp, scale=16.0)
                        return pt

                    def av_tile(kt, pt):
                        if avps[0] is None:
                            avps[0] = ps_av.tile(
                                [128, 512], f32, tag="avps", name="avps"
                            )
                        nc.tensor.matmul(
                            avps[0],
                            lhsT=vsb[:, kt, h, :],
                            rhs=pt,
                            start=(kt == 0),
                            stop=(kt == TT - 1),
                        )

                    pts = [
                        exp_tile(0, s_tile(0)),
                        exp_tile(1, s_tile(1)),
                        exp_tile(2, s_tile(2)),
                    ]
                    for kt in range(3, TT):
                        sp2 = s_tile(kt)
                        av_tile(kt - 3, pts.pop(0))
                        if deferred[0] is not None and 3 <= kt <= 5:
                            emit_norm(deferred[0], kt - 3)
                            if kt == 5:
                                deferred[0] = None
                        pts.append(exp_tile(kt, sp2))
                    for j in range(3):
                        av_tile(TT - 3 + j, pts.pop(0))

                    # avps rows 0:64 hold the softmax row-sums replicated
                    # x64 (ones columns of vsb), rows 64:128 hold A@V.
                    # Scalar does the one cross-partition hop (PSUM 64:128
                    # -> SBUF 0:64); the reciprocal runs at base partition 0
                    # (custom DVE at base 64 corrupts on hardware).
                    uav = puav.tile([64, 512], bf16, tag="uav")
                    rb = prb.tile([64, 512], bf16, tag="rb")
                    deferred[0] = (h, ns, avps[0], uav, rb)

                    # pass 0: deferred rel-h singles as filler (2 per head)
                    if n == 0 and 1 <= h <= 8:
                        relh_single(14 + 2 * h)
                        relh_single(15 + 2 * h)
                    # overlapped projection for pass-0 token blocks, spread
                    # through pass 1 (outT cols 0:512 complete once the
                    # deferred norm of (h=11, n=0) lands during h=0 here);
                    # mt4 pre-accumulates its first 5 head pairs at h=10 so
                    # only its kt=5 matmul waits on the final norm
                    if n == 1 and h in (1, 4, 6, 8):
                        gi = {1: 0, 4: 1, 6: 2, 8: 3}[h]
                        proj_group(mt=gi, g=gi)
                    if n == 1 and h == 10:
                        mt4_tiles = {
                            j: ps_y.tile([128, 384], f32, tag="yps", name="yps4")
                            for j in range(2)
                        }
                        proj_mm(mt4_tiles, 4, range(5))

            # final head's norm: pipelined in column halves
            h, ns_l, avps_l, uav, rb = deferred[0]
            rows = slice(0, 64) if h % 2 == 0 else slice(64, 128)
            for cs, co in ((slice(0, 256), slice(512, 768)),
                           (slice(256, 512), slice(768, 1024))):
                nc.scalar.activation(uav[:, cs], avps_l[64:128, cs], AF.Identity)
                nc.vector._custom_dve(
                    _RF, out=rb[:, cs], in0=avps_l[0:64, cs],
                    s0=_RC["s0"], s1=_RC["s1"], imm2=_RC["imm2"],
                )
                nc.vector.tensor_mul(outT[rows, h // 2, co], uav[:, cs], rb[:, cs])

            # remaining projection: finish mt4, then the last three blocks
            proj_mm(mt4_tiles, 4, range(5, KT))
            proj_finish(mt4_tiles, 4, 4)
            for g, mt in enumerate(range(5, 8)):
                proj_group(mt, 5 + g)

        if dump:
            d_qaug = nc.dram_tensor("d_qaug", [128, NH, T], bf16, kind="ExternalOutput").ap()
            d_kaug = nc.dram_tensor("d_kaug", [128, NH, T], bf16, kind="ExternalOutput").ap()
            d_vsb = nc.dram_tensor("d_vsb", [128, TT, NH, 128], bf16, kind="ExternalOutput").ap()
            d_outT = nc.dram_tensor("d_outT", [128, KT, T], bf16, kind="ExternalOutput").ap()
            nc.sync.dma_start(out=d_qaug, in_=qaug)
            nc.sync.dma_start(out=d_kaug, in_=kaug)
            nc.sync.dma_start(out=d_vsb, in_=vsb)
            nc.sync.dma_start(out=d_outT, in_=outT)

    nc.compile()
    return nc


def _host_consts(qkv_w, proj_w, rel_pos_h, rel_pos_w):
    import ml_dtypes

    bf = ml_dtypes.bfloat16
    f = np.float32
    qs = f(0.125 / 16.0)  # logit scale folded so PSUM holds S/16

    Wq = qkv_w[:, 0:DIM] * qs
    Wk = qkv_w[:, DIM : 2 * DIM]
    wqk = np.empty((NH, 128, KT, 128), dtype=f)
    for h in range(NH):
        m = np.concatenate(
            [Wq[:, h * HD : (h + 1) * HD], Wk[:, h * HD : (h + 1) * HD]], axis=1
        )  # [768, 128]
        wqk[h] = m.reshape(KT, 128, 128).transpose(1, 0, 2)

    wv = np.ascontiguousarray(
        qkv_w[:, 2 * DIM : 3 * DIM].reshape(KT, 128, DIM).transpose(1, 0, 2), dtype=f
    )
    pwt = np.ascontiguousarray(
        proj_w.reshape(KT, 128, DIM).transpose(1, 0, 2), dtype=f
    )

    k_idx = np.arange(T)
    onehot = np.zeros((64, T), dtype=f)
    onehot[k_idx // Ww, k_idx] = 1.0  # rows 0:32  -> h one-hot
    onehot[32 + (k_idx % Ww), k_idx] = 1.0  # rows 32:64 -> w one-hot

    # relh[c, hq, i] = 8 * rel_pos_h[hq - i + (Hh-1), c]; with q scaled by
    # 0.125/16 the matmul yields rel_h/16 exactly like the qk part.
    hq = np.arange(Hh)[:, None]
    ii = np.arange(Hh)[None, :]
    relh = (8.0 * rel_pos_h[(hq - ii + Hh - 1)]).transpose(2, 0, 1)
    relw = (8.0 * rel_pos_w[(hq - ii + Ww - 1)]).transpose(2, 0, 1)
    return {
        "wqk": wqk.astype(bf),
        "wv": wv.astype(bf),
        "pw": pwt.astype(bf),
        "onehot": onehot.astype(bf),
        "relh": np.ascontiguousarray(relh, dtype=f).astype(bf),
        "relw": np.ascontiguousarray(relw, dtype=f).astype(bf),
    }


def _numpy_reference(x, qkv_w, qkv_b, proj_w, proj_b, rel_pos_h, rel_pos_w):
    """Exact fallback (only used if qkv_b's q-part is nonzero)."""
    b, h, w, dim = x.shape
    hw = h * w
    scale = HD ** -0.5
    qkv = x.reshape(b, hw, dim) @ qkv_w + qkv_b
    qkv = qkv.reshape(b, hw, 3, NH, HD).transpose(2, 0, 3, 1, 4)
    qkv = qkv.reshape(3, b * NH, hw, HD)
    q, k, v = qkv[0], qkv[1], qkv[2]
    idx_h = np.arange(h)[:, None] - np.arange(h)[None, :] + (h - 1)
    idx_w = np.arange(w)[:, None] - np.arange(w)[None, :] + (w - 1)
    Rh = rel_pos_h[idx_h]
    Rw = rel_pos_w[idx_w]
    r_q = q.reshape(b * NH, h, w, HD)
    rel_h = np.einsum("bhwc,hkc->bhwk", r_q, Rh)
    rel_w = np.einsum("bhwc,wkc->bhwk", r_q, Rw)
    bias = (rel_h[:, :, :, :, None] + rel_w[:, :, :, None, :]).reshape(
        b * NH, hw, hw
    )
    attn = np.einsum("bqd,bkd->bqk", q, k) * scale + bias
    attn = attn - attn.max(axis=-1, keepdims=True)
    attn = np.exp(attn)
    attn /= attn.sum(axis=-1, keepdims=True)
    out = np.einsum("bqk,bkd->bqd", attn, v)
    out = out.reshape(b, NH, h, w, HD).transpose(0, 2, 3, 1, 4).reshape(b, h, w, dim)
    return (out @ proj_w + proj_b).astype(np.float32)


def kernel(x, qkv_w, qkv_b, proj_w, proj_b, rel_pos_h, rel_pos_w):
    import ml_dtypes

    bf = ml_dtypes.bfloat16
    x = np.asarray(x, dtype=np.float32)
    qkv_w = np.asarray(qkv_w, dtype=np.float32)
    qkv_b = np.asarray(qkv_b, dtype=np.float32)
    proj_w = np.asarray(proj_w, dtype=np.float32)
    proj_b = np.asarray(proj_b, dtype=np.float32)
    rel_pos_h = np.asarray(rel_pos_h, dtype=np.float32)
    rel_pos_w = np.asarray(rel_pos_w, dtype=np.float32)

    if np.any(qkv_b[0:DIM] != 0.0):
        # exact general fallback; never hit for this problem's inputs
        return _numpy_reference(
            x, qkv_w, qkv_b, proj_w, proj_b, rel_pos_h, rel_pos_w
        )

    from concourse.bass_utils import run_bass_kernel_spmd

    nc = _build_program()
    consts = _host_consts(qkv_w, proj_w, rel_pos_h, rel_pos_w)
    in_maps = []
    for b in range(B):
        m = dict(consts)
        m["xT"] = np.ascontiguousarray(x[b].reshape(T, DIM).T).astype(bf)
        in_maps.append(m)

    res = run_bass_kernel_spmd(nc, in_maps, list(range(N_CORES)), trace=TRACE)
    LAST["exec_time_ns"] = res.exec_time_ns
    LAST["results"] = res
    out = np.stack(
        [res.results[b]["y"].astype(np.float32).reshape(Hh, Ww, DIM) for b in range(B)]
    )

    # v-bias + proj-bias contribution (exact; softmax rows sum to 1)
    host_bias = qkv_b[2 * DIM : 3 * DIM] @ proj_w + proj_b
    if np.any(host_bias != 0.0):
        out = out + host_bias.astype(np.float32)
    return out.astype(np.float32, copy=False)
